# revision 1
# baseline (speedup 1.0000x reference)
"""GroupInfoNCE loss kernel for 8 Trainium2 NeuronCores.

Strategy (row-sharded, fused, collective-free):
  - Core k owns rows [1024k, 1024k+1024) of S = scale * f1n @ f2n.T.
  - f2 is passed to core k pre-rotated by -1024k rows so each core's
    diagonal (positive) block sits at local column offset 0; this makes
    the positive-block extraction core-independent (same NEFF on all
    cores, different data).
  - The 8192x8192 logits matrix never touches HBM: each [128,1024] GEMM
    tile is consumed in PSUM by ScalarE exp (with the per-row
    scale*rsqrt(|f1_i|^2) folded into the activation scale AP, so f1 is
    never explicitly normalized) -> bf16 SBUF.
  - Row stats: VectorE 3D-AP reduce gives 16-wide block sums; ScalarE
    Log with accum_out gives log-block-sums + their row sum; exp
    accum_out gives full row sums of exp.
  - Column stats: TensorE ones-matmul accumulates 16-wide column block
    sums over the core's 1024 rows into PSUM [64,1024]; since groups
    partition rows, each core's column block sums are complete -> DMA
    straight PSUM->DRAM. Host does the tiny O(GN) combine.
"""

import os
import numpy as np

GN, D = 8192, 256
NGRP = 16               # group length N
EPS = 0.1               # label smoothing
G = GN // NGRP          # 512 groups
NCORES = 8
RPC = GN // NCORES      # 1024 rows per core
NSTRIP = RPC // 128     # 8 strips of 128 rows
NJB = GN // 1024        # 8 j-blocks of 1024 columns

_cache = {}
last_results = None


def _build_program(ln_s: float, parts: int = 5):
    # parts: 11=loads+stats, 12=+casts, 13=+f1T, 1=full prep,
    #        2=+gemm+exp, 3=+rowred, 4=+colsum, >=5 full
    do_casts = parts >= 12 or parts < 10
    do_f1T = parts == 13 or parts < 10
    do_f2T = parts < 10
    from contextlib import ExitStack
    import concourse.bass as bass  # noqa: F401
    import concourse.mybir as mybir
    import concourse.tile as tile
    from concourse import bacc

    f32 = mybir.dt.float32
    bf16 = mybir.dt.bfloat16
    AF = mybir.ActivationFunctionType
    ALU = mybir.AluOpType
    AX = mybir.AxisListType

    nc = bacc.Bacc(
        "TRN2",
        target_bir_lowering=False,
        debug=False,
        enable_asserts=False,
        num_devices=NCORES,
    )

    f1s_d = nc.dram_tensor("f1s", [RPC, D], f32, kind="ExternalInput").ap()
    f2r_d = nc.dram_tensor("f2r", [GN, D], f32, kind="ExternalInput").ap()
    ones64_d = nc.dram_tensor(
        "ones64", [128, NSTRIP, 64], bf16, kind="ExternalInput"
    ).ap()
    mask128_d = nc.dram_tensor("mask128", [128, 8], f32, kind="ExternalInput").ap()

    o_asum_d = nc.dram_tensor("o_asum", [128, NSTRIP], f32, kind="ExternalOutput").ap()
    o_slog_d = nc.dram_tensor("o_slog", [128, NSTRIP], f32, kind="ExternalOutput").ap()
    o_pos_d = nc.dram_tensor("o_pos", [128, NSTRIP], f32, kind="ExternalOutput").ap()
    o_craw_d = nc.dram_tensor("o_craw", [64, GN], f32, kind="ExternalOutput").ap()

    with tile.TileContext(nc) as tc, ExitStack() as ctx:
        singles = ctx.enter_context(tc.tile_pool(name="singles", bufs=1))
        stage = ctx.enter_context(tc.tile_pool(name="stage", bufs=6))
        scratch = ctx.enter_context(tc.tile_pool(name="scratch", bufs=2))
        expp = ctx.enter_context(tc.tile_pool(name="expp", bufs=4))
        psg = ctx.enter_context(tc.tile_pool(name="psg", bufs=2, space="PSUM"))
        psc = ctx.enter_context(tc.tile_pool(name="psc", bufs=2, space="PSUM"))

        ones64_sb = singles.tile([128, NSTRIP, 64], bf16, name="ones64_sb")
        nc.sync.dma_start(out=ones64_sb, in_=ones64_d)
        mask128_sb = singles.tile([128, 8], f32, name="mask128_sb")
        nc.sync.dma_start(out=mask128_sb, in_=mask128_d)

        f1T = [
            singles.tile([128, RPC], bf16, name=f"f1T{h}", tag=f"f1T{h}")
            for h in (0, 1)
        ] if do_f1T else None
        f2T = [
            singles.tile([128, GN], bf16, name=f"f2T{h}", tag=f"f2T{h}")
            for h in (0, 1)
        ] if do_f2T else None
        ssq1 = singles.tile([128, NSTRIP], f32, name="ssq1")
        lssq = singles.tile([128, NSTRIP], f32, name="lssq")
        a_scale = singles.tile([128, NSTRIP], f32, name="a_scale")
        rowblk = [
            singles.tile([128, G], f32, name=f"rowblk{t}", tag=f"rowblk{t}")
            for t in range(NSTRIP)
        ] if 3 <= parts < 10 else None
        easum = [
            singles.tile([128, NJB], f32, name=f"easum{t}", tag=f"easum{t}")
            for t in range(NSTRIP)
        ] if 2 <= parts < 10 else None
        if 5 <= parts < 10:
            o_asum_sb = singles.tile([128, NSTRIP], f32, name="o_asum_sb")
            o_slog_sb = singles.tile([128, NSTRIP], f32, name="o_slog_sb")
            o_pos_sb = singles.tile([128, NSTRIP], f32, name="o_pos_sb")

        # ---------------- f1 prep: sumsq + bf16 cast + transpose ----------------
        for t in range(NSTRIP):
            f1tile = stage.tile([128, D], f32, tag="f1in", name="f1tile")
            nc.gpsimd.dma_start(out=f1tile, in_=f1s_d[t * 128 : (t + 1) * 128, :])
            sq = scratch.tile([128, D], bf16, tag="sq", name="sq")
            nc.scalar.activation(
                sq, f1tile, AF.Square, accum_out=ssq1[:, t : t + 1]
            )
            if do_casts:
                f1b = stage.tile([128, D], bf16, tag="f1b", name="f1b")
                nc.scalar.copy(f1b, f1tile)
                if do_f1T:
                    for h in (0, 1):
                        eng = nc.sync if h == 0 else nc.scalar
                        eng.dma_start_transpose(
                            f1T[h][:, t * 128 : (t + 1) * 128],
                            f1b[:, h * 128 : (h + 1) * 128],
                        )
        # a_scale[:, t] = s * rsqrt(ssq1[:, t]) = exp(-0.5*log(ssq) + ln(s))
        lns_sb = singles.tile([128, 1], f32, name="lns_sb")
        nc.vector.memset(lns_sb, ln_s)
        zero_sb = singles.tile([128, 1], f32, name="zero_sb")
        nc.vector.memset(zero_sb, 0.0)
        nc.scalar.activation(lssq, ssq1, AF.Ln, bias=zero_sb)
        nc.scalar.activation(a_scale, lssq, AF.Exp, scale=-0.5, bias=lns_sb)

        # ---------------- f2 prep: normalize + bf16 cast + transpose ------------
        for ft in range(GN // 128):
            f2tile = stage.tile([128, D], f32, tag="f2in", name="f2tile")
            nc.gpsimd.dma_start(out=f2tile, in_=f2r_d[ft * 128 : (ft + 1) * 128, :])
            sq2 = scratch.tile([128, D], bf16, tag="sq", name="sq2")
            ssq2 = scratch.tile([128, 1], f32, tag="ssq2", name="ssq2")
            nc.scalar.activation(sq2, f2tile, AF.Square, accum_out=ssq2)
            lg2 = scratch.tile([128, 1], f32, tag="lg2", name="lg2")
            nc.scalar.activation(lg2, ssq2, AF.Ln, bias=zero_sb)
            rs2 = scratch.tile([128, 1], f32, tag="rs2", name="rs2")
            nc.scalar.activation(rs2, lg2, AF.Exp, scale=-0.5, bias=zero_sb)
            if do_casts:
                f2b = stage.tile([128, D], bf16, tag="f2b", name="f2b")
                nc.vector.tensor_scalar_mul(f2b, f2tile, rs2)
                if do_f2T:
                    for h in (0, 1):
                        eng = nc.sync if h == 0 else nc.scalar
                        eng.dma_start_transpose(
                            f2T[h][:, ft * 128 : (ft + 1) * 128],
                            f2b[:, h * 128 : (h + 1) * 128],
                        )

        # ---------------- main fused GEMM + stats loop --------------------------
        for jb in range(NJB if 2 <= parts < 10 else 0):
            colps = psc.tile([64, 1024], f32, tag="colps", name="colps") if 4 <= parts < 10 else None
            for t in range(NSTRIP):
                ps = psg.tile([128, 1024], f32, tag="gemm", name="ps")
                for h in (0, 1):
                    for kc in (0, 1):
                        nc.tensor.matmul(
                            ps[:, h * 512 : (h + 1) * 512],
                            lhsT=f1T[kc][:, t * 128 : (t + 1) * 128],
                            rhs=f2T[kc][
                                :, jb * 1024 + h * 512 : jb * 1024 + (h + 1) * 512
                            ],
                            start=(kc == 0),
                            stop=(kc == 1),
                        )
                expb = expp.tile([128, 1024], bf16, tag="exp", name="expb")
                nc.scalar.activation(
                    expb,
                    ps,
                    AF.Exp,
                    bias=zero_sb,
                    scale=a_scale[:, t : t + 1],
                    accum_out=easum[t][:, jb : jb + 1],
                )
                if 3 <= parts < 10:
                    nc.vector.reduce_sum(
                        out=rowblk[t][:, jb * 64 : (jb + 1) * 64],
                        in_=expb.rearrange("p (g n) -> p g n", n=NGRP),
                        axis=AX.X,
                    )
                if 4 <= parts < 10:
                    for h in (0, 1):
                        nc.tensor.matmul(
                            colps[:, h * 512 : (h + 1) * 512],
                            lhsT=ones64_sb[:, t, :],
                            rhs=expb[:, h * 512 : (h + 1) * 512],
                            start=(t == 0),
                            stop=(t == NSTRIP - 1),
                        )
            if 4 <= parts < 10:
                craw_sb = expp.tile([64, 1024], f32, tag="craw_sb", name="craw_sb")
                nc.vector.tensor_copy(craw_sb, colps)
                nc.gpsimd.dma_start(
                    out=o_craw_d[:, jb * 1024 : (jb + 1) * 1024], in_=craw_sb
                )

        # ---------------- per-strip tails ---------------------------------------
        for t in range(NSTRIP if 5 <= parts < 10 else 0):
            nc.vector.reduce_sum(
                out=o_asum_sb[:, t : t + 1], in_=easum[t], axis=AX.X
            )
            nc.scalar.activation(
                rowblk[t], rowblk[t], AF.Ln, bias=zero_sb,
                accum_out=o_slog_sb[:, t : t + 1],
            )
            posscr = scratch.tile([128, 8], f32, tag="posscr", name="posscr")
            nc.vector.tensor_mul(
                posscr, rowblk[t][:, t * 8 : (t + 1) * 8], mask128_sb
            )
            nc.vector.reduce_sum(
                out=o_pos_sb[:, t : t + 1], in_=posscr, axis=AX.X
            )
        if 5 <= parts < 10:
            nc.gpsimd.dma_start(out=o_asum_d, in_=o_asum_sb)
            nc.gpsimd.dma_start(out=o_slog_d, in_=o_slog_sb)
            nc.gpsimd.dma_start(out=o_pos_d, in_=o_pos_sb)

    nc.compile()
    return nc


def _constants():
    import ml_dtypes

    p = np.arange(128)
    ones64 = np.zeros((128, NSTRIP, 64), dtype=ml_dtypes.bfloat16)
    for t in range(NSTRIP):
        ones64[p, t, 8 * t + p // 16] = 1.0
    mask128 = np.zeros((128, 8), dtype=np.float32)
    mask128[p, p // 16] = 1.0
    return ones64, mask128


def kernel(image_features1, image_features2, logit_scale):
    global last_results
    from concourse.bass_utils import run_bass_kernel_spmd

    f1 = np.ascontiguousarray(np.asarray(image_features1, dtype=np.float32))
    f2 = np.ascontiguousarray(np.asarray(image_features2, dtype=np.float32))
    s = float(np.asarray(logit_scale).reshape(-1)[0])

    key = round(np.log(s), 12)
    if key not in _cache:
        _cache[key] = _build_program(float(np.log(s)))
    nc = _cache[key]

    ones64, mask128 = _constants()
    in_maps = []
    for k in range(NCORES):
        in_maps.append(
            {
                "f1s": np.ascontiguousarray(f1[k * RPC : (k + 1) * RPC]),
                "f2r": np.ascontiguousarray(np.roll(f2, -k * RPC, axis=0)),
                "ones64": ones64,
                "mask128": mask128,
            }
        )

    try:
        res = run_bass_kernel_spmd(
            nc,
            in_maps,
            core_ids=list(range(NCORES)),
            trace=bool(os.environ.get("KTRACE")),
        )
    except ModuleNotFoundError:
        # axon build without NTFF profiling hooks — rerun without trace
        res = run_bass_kernel_spmd(
            nc, in_maps, core_ids=list(range(NCORES)), trace=False
        )
    last_results = res

    # ---------------- host combine (O(GN) work) ----------------
    eps = EPS
    S1 = 0.0
    for k in range(NCORES):
        r = res.results[k]
        asum = r["o_asum"].astype(np.float64)  # [128, NSTRIP] sum_j exp
        slog = r["o_slog"].astype(np.float64)  # [128, NSTRIP] sum_g log blocksum
        pos = r["o_pos"].astype(np.float64)  # [128, NSTRIP] log blocksum at pos
        per_row = np.log(asum) - (1.0 - eps) * pos - (eps / G) * slog
        S1 += per_row.sum()

    j = np.arange(GN)
    a_tot = np.zeros(GN, dtype=np.float64)
    b_tot = np.zeros(GN, dtype=np.float64)
    pos2 = np.zeros(GN, dtype=np.float64)
    for k in range(NCORES):
        craw = res.results[k]["o_craw"].astype(np.float64)  # [64, GN] local cols
        jj = (j - k * RPC) % GN
        cg = craw[:, jj]  # columns reindexed to global j
        a_tot += cg.sum(axis=0)
        b_tot += np.log(cg).sum(axis=0)
        jr = np.arange(k * RPC, (k + 1) * RPC)
        pos2[jr] = craw[(jr // 16) % 64, jr % RPC]
    per_row2 = np.log(a_tot) - (1.0 - eps) * np.log(pos2) - (eps / G) * b_tot
    S2 = per_row2.sum()

    loss = (S1 + S2) / (2.0 * GN)
    return np.array(loss, dtype=np.float32)



# revision 9
# speedup vs baseline: 1.0682x; 1.0682x over previous
"""GroupInfoNCE loss kernel for 8 Trainium2 NeuronCores.

Strategy (row-sharded, fused, collective-free):
  - Core k owns rows [1024k, 1024k+1024) of S = scale * f1n @ f2n.T.
  - f2 is passed to core k pre-rotated by -1024k rows so each core's
    diagonal (positive) block sits at local column offset 0; this makes
    the positive-block extraction core-independent (same NEFF on all
    cores, different data).
  - The 8192x8192 logits matrix never touches HBM: each [128,1024] GEMM
    tile is consumed in PSUM by ScalarE exp (with the per-row
    scale*rsqrt(|f1_i|^2) folded into the activation scale AP, so f1 is
    never explicitly normalized) -> bf16 SBUF.
  - Engine balance: square+sumsq on DVE (tensor_tensor_reduce), bf16
    casts on GpSimd/Pool, exp + log on ScalarE pinned to the
    natural_log_exp activation table (one explicit LoadActFuncSet, no
    per-activation table reloads), GEMM + column block sums on PE,
    row block sums on DVE.
  - Loads/transposes batched into few DMA instructions issued from SP
    (HWDGE); column sums DMA'd straight PSUM->DRAM. Host does the tiny
    O(GN) combine.
"""

import os
import numpy as np

GN, D = 8192, 256
NGRP = 16               # group length N
EPS = 0.1               # label smoothing
G = GN // NGRP          # 512 groups
NCORES = 8
RPC = GN // NCORES      # 1024 rows per core
NSTRIP = RPC // 128     # 8 strips of 128 rows
NJB = GN // 1024        # 8 j-blocks of 1024 columns
NCH = GN // 128         # 64 f2 chunks of 128 rows

ACT_TABLE_LN_EXP = 6    # act_info.json index of natural_log_exp_and_others

_cache = {}
last_results = None


def _build_program(ln_s: float, parts: int = 5, repeat: int = 1):
    # parts: 1=loads+prep, 2=+gemm+exp, 3=+rowred, 4=+colsum, >=5 full
    # fine-grain prep bisect: 11=loads, 12=+ttr, 13=+scales, 14=+casts,
    # 15=+transposes (15 == 1). parts>=20: like parts-19 but skip the
    # explicit LoadActFuncSet.
    skip_actload = parts >= 20
    if skip_actload:
        parts -= 19
    do_ttr = parts >= 12 or parts < 10
    do_scales = parts >= 13 or parts < 10
    do_casts = parts >= 14 or parts < 10
    do_transp = parts >= 15 or parts < 10
    from contextlib import ExitStack
    import concourse.bass as bass  # noqa: F401
    import concourse.mybir as mybir
    import concourse.tile as tile
    from concourse import bacc

    f32 = mybir.dt.float32
    bf16 = mybir.dt.bfloat16
    AF = mybir.ActivationFunctionType
    ALU = mybir.AluOpType
    AX = mybir.AxisListType

    nc = bacc.Bacc(
        "TRN2",
        target_bir_lowering=False,
        debug=False,
        enable_asserts=False,
        num_devices=NCORES,
    )

    f1s_d = nc.dram_tensor("f1s", [RPC, D], f32, kind="ExternalInput").ap()
    f2r_d = nc.dram_tensor("f2r", [GN, D], f32, kind="ExternalInput").ap()
    ones64_d = nc.dram_tensor(
        "ones64", [128, NSTRIP, 64], bf16, kind="ExternalInput"
    ).ap()
    mask128_d = nc.dram_tensor("mask128", [128, 8], f32, kind="ExternalInput").ap()

    # o_small[:, 0, :]=asum  [:, 1, :]=slog  [:, 2, :]=pos
    o_small_d = nc.dram_tensor("o_small", [128, 3, NSTRIP], f32, kind="ExternalOutput").ap()
    o_craw_d = nc.dram_tensor("o_craw", [64, GN], f32, kind="ExternalOutput").ap()

    with tile.TileContext(nc) as tc, ExitStack() as ctx:
        singles = ctx.enter_context(tc.tile_pool(name="singles", bufs=1))
        sqp = ctx.enter_context(tc.tile_pool(name="sqp", bufs=2))
        expp = ctx.enter_context(tc.tile_pool(name="expp", bufs=4))
        psg = ctx.enter_context(tc.tile_pool(name="psg", bufs=2, space="PSUM"))
        psc = ctx.enter_context(tc.tile_pool(name="psc", bufs=2, space="PSUM"))

        ones64_sb = singles.tile([128, NSTRIP, 64], bf16, name="ones64_sb")
        mask128_sb = singles.tile([128, 8], f32, name="mask128_sb")

        f1all = singles.tile([128, NSTRIP, D], f32, name="f1all")
        f2all = [
            singles.tile([128, 16, D], f32, name=f"f2all{q}", tag=f"f2all{q}")
            for q in range(4)
        ]
        f1bh = [
            singles.tile([128, NSTRIP, 128], bf16, name=f"f1bh{h}", tag=f"f1bh{h}")
            for h in (0, 1)
        ]
        f2bh = [
            [
                singles.tile([128, 8, 128], bf16, name=f"f2bh{jb}_{h}", tag=f"f2bh{jb}_{h}")
                for h in (0, 1)
            ]
            for jb in range(NJB)
        ]
        f1T = [
            singles.tile([128, RPC], bf16, name=f"f1T{h}", tag=f"f1T{h}")
            for h in (0, 1)
        ]
        f2T = [
            [
                singles.tile([128, 8, 128], bf16, name=f"f2T{jb}_{h}", tag=f"f2T{jb}_{h}")
                for h in (0, 1)
            ]
            for jb in range(NJB)
        ]
        ssq_all = singles.tile([128, NSTRIP + NCH], f32, name="ssq_all")
        lssq_all = singles.tile([128, NSTRIP + NCH], f32, name="lssq_all")
        scales = singles.tile([128, NSTRIP + NCH], f32, name="scales")
        lns_sb = singles.tile([128, 1], f32, name="lns_sb")
        rowblk = [
            singles.tile([128, G], f32, name=f"rowblk{t}", tag=f"rowblk{t}")
            for t in range(NSTRIP)
        ]
        easum = [
            singles.tile([128, NJB], f32, name=f"easum{t}", tag=f"easum{t}")
            for t in range(NSTRIP)
        ]
        o_small_sb = singles.tile([128, 3, NSTRIP], f32, name="o_small_sb")

        nc.sync.dma_start(out=ones64_sb, in_=ones64_d)
        nc.sync.dma_start(out=mask128_sb, in_=mask128_d)
        nc.vector.memset(lns_sb, ln_s)

        # pin the one activation table that covers exp/ln/square/copy
        if not skip_actload:
            nc.scalar.add_instruction(mybir.InstLoadActFuncSet(
                name=nc.get_next_instruction_name(),
                act_func_set_id=ACT_TABLE_LN_EXP, ins=[], outs=[]))

        for _rep in range(repeat):
            # ---------------- loads (SP / HWDGE, few big DMAs) ----------------
            nc.sync.dma_start(
                out=f1all, in_=f1s_d.rearrange("(t p) d -> p t d", p=128)
            )
            for q in range(4):
                nc.sync.dma_start(
                    out=f2all[q],
                    in_=f2r_d[q * 2048 : (q + 1) * 2048, :].rearrange(
                        "(t p) d -> p t d", p=128
                    ),
                )

            # ---------------- f1 prep: sumsq (DVE) + cast (Pool) --------------
            for t in range(NSTRIP):
                if do_ttr:
                    sq = sqp.tile([128, D], f32, tag="sq", name="sq")
                    nc.gpsimd.tensor_mul(sq, f1all[:, t, :], f1all[:, t, :])
                    nc.vector.reduce_sum(
                        out=ssq_all[:, t : t + 1], in_=sq, axis=AX.X
                    )
                if do_casts:
                    for h in (0, 1):
                        nc.gpsimd.tensor_copy(
                            f1bh[h][:, t, :], f1all[:, t, h * 128 : (h + 1) * 128]
                        )
            # ---------------- f2 sumsq (DVE) ----------------------------------
            for c in range(NCH if do_ttr else 0):
                sq2 = sqp.tile([128, D], f32, tag="sq", name="sq2")
                nc.gpsimd.tensor_mul(
                    sq2,
                    f2all[c // 16][:, c % 16, :],
                    f2all[c // 16][:, c % 16, :],
                )
                nc.vector.reduce_sum(
                    out=ssq_all[:, NSTRIP + c : NSTRIP + c + 1], in_=sq2, axis=AX.X
                )
            # scales = exp(-0.5*ln(ssq) + bias); bias=ln_s for f1 else 0
            if do_scales:
                nc.scalar.activation(lssq_all, ssq_all, AF.Ln)
                nc.scalar.activation(
                    scales[:, :NSTRIP], lssq_all[:, :NSTRIP], AF.Exp,
                    scale=-0.5, bias=lns_sb,
                )
                nc.scalar.activation(
                    scales[:, NSTRIP:], lssq_all[:, NSTRIP:], AF.Exp, scale=-0.5
                )

            # ---------------- f1 transpose (2 DMA-transpose instrs) -----------
            for h in (0, 1) if do_transp else ():
                nc.sync.dma_start_transpose(
                    f1T[h].rearrange("p (t j) -> p t j", j=128), f1bh[h]
                )

            # ---------------- f2 cast+scale (Pool) + transpose per jb ---------
            for jb in range(NJB if do_casts else 0):
                for cc in range(8):
                    c = jb * 8 + cc
                    for h in (0, 1):
                        nc.gpsimd.tensor_scalar_mul(
                            f2bh[jb][h][:, cc, :],
                            f2all[c // 16][:, c % 16, h * 128 : (h + 1) * 128],
                            scales[:, NSTRIP + c : NSTRIP + c + 1],
                        )
                if do_transp:
                    for h in (0, 1):
                        nc.sync.dma_start_transpose(f2T[jb][h], f2bh[jb][h])

            # ---------------- main fused GEMM + stats loop --------------------
            for jb in range(NJB if 2 <= parts < 10 else 0):
                colps = psc.tile([64, 1024], f32, tag="colps", name="colps") if parts >= 4 else None
                rhs = [
                    f2T[jb][h].rearrange("p t j -> p (t j)") for h in (0, 1)
                ]
                for t in range(NSTRIP):
                    ps = psg.tile([128, 1024], f32, tag="gemm", name="ps")
                    for h2 in (0, 1):
                        for kc in (0, 1):
                            nc.tensor.matmul(
                                ps[:, h2 * 512 : (h2 + 1) * 512],
                                lhsT=f1T[kc][:, t * 128 : (t + 1) * 128],
                                rhs=rhs[kc][:, h2 * 512 : (h2 + 1) * 512],
                                start=(kc == 0),
                                stop=(kc == 1),
                            )
                    expb = expp.tile([128, 1024], bf16, tag="exp", name="expb")
                    nc.scalar.activation(
                        expb, ps, AF.Exp,
                        scale=scales[:, t : t + 1],
                        accum_out=easum[t][:, jb : jb + 1],
                    )
                    if parts >= 3:
                        nc.vector.reduce_sum(
                            out=rowblk[t][:, jb * 64 : (jb + 1) * 64],
                            in_=expb.rearrange("p (g n) -> p g n", n=NGRP),
                            axis=AX.X,
                        )
                    if parts >= 4:
                        for h2 in (0, 1):
                            nc.tensor.matmul(
                                colps[:, h2 * 512 : (h2 + 1) * 512],
                                lhsT=ones64_sb[:, t, :],
                                rhs=expb[:, h2 * 512 : (h2 + 1) * 512],
                                start=(t == 0),
                                stop=(t == NSTRIP - 1),
                            )
                if parts >= 4:
                    craw_sb = expp.tile([64, 1024], f32, tag="craw_sb", name="craw_sb")
                    nc.scalar.copy(craw_sb, colps)
                    nc.sync.dma_start(
                        out=o_craw_d[:, jb * 1024 : (jb + 1) * 1024], in_=craw_sb
                    )

            # ---------------- per-strip tails ---------------------------------
            for t in range(NSTRIP if 5 <= parts < 10 else 0):
                nc.vector.reduce_sum(
                    out=o_small_sb[:, 0, t : t + 1], in_=easum[t], axis=AX.X
                )
                nc.scalar.activation(
                    rowblk[t], rowblk[t], AF.Ln,
                    accum_out=o_small_sb[:, 1, t : t + 1],
                )
                posscr = sqp.tile([128, 8], f32, tag="posscr", name="posscr")
                nc.gpsimd.tensor_mul(
                    posscr, rowblk[t][:, t * 8 : (t + 1) * 8], mask128_sb
                )
                nc.vector.reduce_sum(
                    out=o_small_sb[:, 2, t : t + 1], in_=posscr, axis=AX.X
                )
            if 5 <= parts < 10:
                nc.sync.dma_start(out=o_small_d, in_=o_small_sb)

    nc.compile()
    return nc


def _constants():
    import ml_dtypes

    p = np.arange(128)
    ones64 = np.zeros((128, NSTRIP, 64), dtype=ml_dtypes.bfloat16)
    for t in range(NSTRIP):
        ones64[p, t, 8 * t + p // 16] = 1.0
    mask128 = np.zeros((128, 8), dtype=np.float32)
    mask128[p, p // 16] = 1.0
    return ones64, mask128


def make_in_maps(f1, f2):
    ones64, mask128 = _constants()
    return [
        {
            "f1s": np.ascontiguousarray(f1[k * RPC : (k + 1) * RPC]),
            "f2r": np.ascontiguousarray(np.roll(f2, -k * RPC, axis=0)),
            "ones64": ones64,
            "mask128": mask128,
        }
        for k in range(NCORES)
    ]


def kernel(image_features1, image_features2, logit_scale):
    global last_results
    from concourse.bass_utils import run_bass_kernel_spmd

    f1 = np.ascontiguousarray(np.asarray(image_features1, dtype=np.float32))
    f2 = np.ascontiguousarray(np.asarray(image_features2, dtype=np.float32))
    s = float(np.asarray(logit_scale).reshape(-1)[0])

    key = round(np.log(s), 12)
    if key not in _cache:
        _cache[key] = _build_program(float(np.log(s)))
    nc = _cache[key]

    in_maps = make_in_maps(f1, f2)

    try:
        res = run_bass_kernel_spmd(
            nc,
            in_maps,
            core_ids=list(range(NCORES)),
            trace=bool(os.environ.get("KTRACE")),
        )
    except ModuleNotFoundError:
        # axon build without NTFF profiling hooks — rerun without trace
        res = run_bass_kernel_spmd(
            nc, in_maps, core_ids=list(range(NCORES)), trace=False
        )
    last_results = res

    # ---------------- host combine (O(GN) work) ----------------
    eps = EPS
    S1 = 0.0
    for k in range(NCORES):
        r = res.results[k]
        small = r["o_small"].astype(np.float64)  # [128, 3, NSTRIP]
        asum = small[:, 0, :]  # sum_j exp
        slog = small[:, 1, :]  # sum_g log blocksum
        pos = small[:, 2, :]   # log blocksum at positive block
        per_row = np.log(asum) - (1.0 - eps) * pos - (eps / G) * slog
        S1 += per_row.sum()

    j = np.arange(GN)
    a_tot = np.zeros(GN, dtype=np.float64)
    b_tot = np.zeros(GN, dtype=np.float64)
    pos2 = np.zeros(GN, dtype=np.float64)
    for k in range(NCORES):
        craw = res.results[k]["o_craw"].astype(np.float64)  # [64, GN] local cols
        jj = (j - k * RPC) % GN
        cg = craw[:, jj]  # columns reindexed to global j
        a_tot += cg.sum(axis=0)
        b_tot += np.log(cg).sum(axis=0)
        jr = np.arange(k * RPC, (k + 1) * RPC)
        pos2[jr] = craw[(jr // 16) % 64, jr % RPC]
    per_row2 = np.log(a_tot) - (1.0 - eps) * np.log(pos2) - (eps / G) * b_tot
    S2 = per_row2.sum()

    loss = (S1 + S2) / (2.0 * GN)
    return np.array(loss, dtype=np.float32)
